# revision 14
# baseline (speedup 1.0000x reference)
"""Trainium2 Bass kernel for nn_Base_75265006895876 (retrieval_knn).

Data-parallel over batch B=128 -> 16 per core on 8 NeuronCores; the
cap_embedding table is replicated per core.  Per core, (t, b) pairs are
processed in groups of 8 pairs (= 128 gathered embedding rows):

  indirect-DMA gather rows (fp32 -> bf16 cast in DMA)
  -> 8x HWDGE xbar dma_start_transpose (SBUF->SBUF, D onto partitions)
  -> PE gram+dots matmuls (bf16, fp32 accum), fused masking matmuls
  -> cheap per-group column extractions into [128, NGROUPS] tiles
  -> one batched tail: norms / min-distance / cosine chains
  -> results tile -> single DMA out
"""

import sys

if "/opt/trn_rl_repo" not in sys.path:
    sys.path.insert(0, "/opt/trn_rl_repo")

import numpy as np

# ---- problem constants (hardcoded; kernel.py must be self-contained) ----
T, B, K, L, V, D = 17, 128, 16, 24, 30000, 1024
NCORES = 8
BL = B // NCORES              # 16 local batch rows per core
PAIRS = T * BL                # 272 (t, b) pairs per core
NG = PAIRS * K // 128         # 34 groups of 128 gathered rows
PPG = 128 // K                # 8 pairs per group
LPAD = 32                     # caption length padded 24 -> 32
NCAP = BL * LPAD // 128       # 4 caption gather groups
CH = D // 128                 # 8 contraction chunks of 128
CW = 128 + K                  # 144: chunk cols + sentence cols in tb
LARGE = 1.0e6

_CACHE = {}


def _build_nc():
    from concourse import bass, bacc, mybir

    f32 = mybir.dt.float32
    bf16 = mybir.dt.bfloat16
    AF = mybir.ActivationFunctionType
    ALU = mybir.AluOpType

    nc = bacc.Bacc("TRN2", debug=False)

    table = nc.dram_tensor("table", [V, D], f32, kind="ExternalInput")
    idx_topk_d = nc.dram_tensor("idx_topk", [128, NG], mybir.dt.int32,
                                kind="ExternalInput")
    idx_cap_d = nc.dram_tensor("idx_cap", [128, NCAP], mybir.dt.int32,
                               kind="ExternalInput")
    maskB_d = nc.dram_tensor("maskB", [128, NCAP * BL], bf16,
                             kind="ExternalInput")
    i128_d = nc.dram_tensor("i128", [128, 128], f32, kind="ExternalInput")
    i128b_d = nc.dram_tensor("i128b", [128, 128], bf16, kind="ExternalInput")
    ineg_d = nc.dram_tensor("ineg", [128, 128], bf16, kind="ExternalInput")
    cmask_d = nc.dram_tensor("cmask", [128, 128], bf16, kind="ExternalInput")
    w0_d = nc.dram_tensor("w0", [128, 128], f32, kind="ExternalInput")
    m0p_d = nc.dram_tensor("m0p", [128, NG * CH], f32, kind="ExternalInput")
    mdot_d = nc.dram_tensor("mdot", [128, NG * K], f32, kind="ExternalInput")

    res_d = nc.dram_tensor("res", [128, 3 * NG], f32, kind="ExternalOutput")

    from concourse.tile import TileContext
    from contextlib import ExitStack

    with ExitStack() as ctx:
        tc = ctx.enter_context(TileContext(nc))
        cp = ctx.enter_context(tc.tile_pool(name="cp", bufs=1))
        xp = ctx.enter_context(tc.tile_pool(name="xp", bufs=4))
        cxp = ctx.enter_context(tc.tile_pool(name="cxp", bufs=2))
        smp = ctx.enter_context(tc.tile_pool(name="smp", bufs=3))
        ptp = ctx.enter_context(tc.tile_pool(name="ptp", bufs=3, space="PSUM"))
        pwp = ctx.enter_context(tc.tile_pool(name="pwp", bufs=5, space="PSUM"))

        # ---- load constants / indices into SBUF ----
        c_i128 = cp.tile([128, 128], f32)
        nc.sync.dma_start(c_i128[:], i128_d[:])
        c_i128b = cp.tile([128, 128], bf16)
        nc.sync.dma_start(c_i128b[:], i128b_d[:])
        c_ineg = cp.tile([128, 128], bf16)
        nc.sync.dma_start(c_ineg[:], ineg_d[:])
        c_cm = cp.tile([128, 128], bf16)
        nc.sync.dma_start(c_cm[:], cmask_d[:])
        c_w0 = cp.tile([128, 128], f32)
        nc.sync.dma_start(c_w0[:], w0_d[:])
        c_m0p = cp.tile([128, NG * CH], f32)
        nc.sync.dma_start(c_m0p[:], m0p_d[:])
        c_mdot = cp.tile([128, NG * K], f32)
        nc.sync.dma_start(c_mdot[:], mdot_d[:])
        c_maskB = cp.tile([128, NCAP * BL], bf16)
        nc.sync.dma_start(c_maskB[:], maskB_d[:])
        c_idxt = cp.tile([128, NG], mybir.dt.int32)
        nc.sync.dma_start(c_idxt[:], idx_topk_d[:])
        c_idxc = cp.tile([128, NCAP], mybir.dt.int32)
        nc.sync.dma_start(c_idxc[:], idx_cap_d[:])

        res_sb = cp.tile([128, 3 * NG], f32)

        # per-group accumulation tiles for the batched tail
        sq_all = cp.tile([128, NG], f32)
        sqb_all = cp.tile([128, NG], bf16)
        mx_all = cp.tile([128, NG], f32)
        g0c_all = cp.tile([128, NG * CH], f32)
        dot_all = cp.tile([128, NG * K], f32)

        # ---- emission helpers ----
        def emit_gather_transpose(g):
            """stage 1: gather group g, transpose its 8 chunks, row norms^2."""
            x = xp.tile([128, D], bf16, tag="xg", name=f"x{g}", uniquify=False)
            nc.gpsimd.indirect_dma_start(
                out=x[:], out_offset=None, in_=table[:],
                in_offset=bass.IndirectOffsetOnAxis(
                    ap=c_idxt[:, g:g + 1], axis=0),
            )
            tp = ptp.tile([128, D], bf16, space="PSUM", tag="tp",
                          name=f"tp{g}", uniquify=False)
            for c in range(CH):
                nc.tensor.transpose(
                    tp[:, 128 * c:128 * (c + 1)],
                    x[:, 128 * c:128 * (c + 1)],
                    c_i128b[:],
                )
            xsq_scr = smp.tile([128, D], bf16, tag="xsq", name=f"xs{g}")
            nc.scalar.activation(xsq_scr[:], x[:], AF.Square,
                                 accum_out=sq_all[:, g:g + 1])
            return tp

        def emit_group(g, tp):
            """stage 2: copies, gram+dots matmuls, extractions, masking."""
            tb = tb_tiles[g % 3]
            dst = tb[:].rearrange("p (c w) -> p c w", w=CW)[:, :, 0:128]
            src = tp[:].rearrange("p (c w) -> p c w", w=128)
            nc.vector.tensor_copy(dst[:, 0:5], src[:, 0:5])
            nc.scalar.copy(dst[:, 5:], src[:, 5:])

            wk = pwp.tile([128, 512], f32, space="PSUM", tag="wk",
                          name=f"wk{g}")
            for c in range(CH):
                nc.tensor.matmul(
                    wk[:, 0:CW],
                    lhsT=tb[:, CW * c:CW * c + 128],
                    rhs=tb[:, CW * c:CW * c + CW],
                    start=(c == 0), stop=False,
                    skip_group_check=True,
                )
            nc.vector.tensor_copy(sqb_all[:, g:g + 1], sq_all[:, g:g + 1])
            gv = wk[:, 0:128].rearrange("p (a b) -> p a b", b=K)[:, :, 0:1]
            nc.vector.tensor_copy(
                g0c_all[:, CH * g:CH * (g + 1)].rearrange(
                    "p (a b) -> p a b", b=1), gv)
            nc.vector.tensor_copy(dot_all[:, K * g:K * (g + 1)],
                                  wk[:, 128:128 + K])
            nc.tensor.matmul(
                wk[:, 0:128], lhsT=sqb_all[:, g:g + 1].to_broadcast([128, 128]),
                rhs=c_ineg[:], start=False, stop=False,
                skip_group_check=True,
            )
            nc.tensor.matmul(
                wk[:, 0:128], lhsT=c_cm[:], rhs=c_i128b[:],
                start=False, stop=True, skip_group_check=True,
            )
            nc.vector.tensor_reduce(mx_all[:, g:g + 1], wk[:, 0:128],
                                    axis=mybir.AxisListType.X,
                                    op=ALU.max)

        def emit_tail(lo, hi, part):
            n = hi - lo
            sl = slice(lo, hi)
            # y = 2*sq - bf16(sq): exact cancellation for duplicate rows
            sqbf = cp.tile([128, n], f32, name=f"sqbf{part}")
            nc.vector.tensor_copy(sqbf[:], sqb_all[:, sl])
            y_all = cp.tile([128, n], f32, name=f"y{part}")
            nc.vector.tensor_scalar(out=y_all[:], in0=sq_all[:, sl],
                                    scalar1=2.0, scalar2=None, op0=ALU.mult)
            nc.vector.tensor_tensor(out=y_all[:], in0=y_all[:], in1=sqbf[:],
                                    op=ALU.subtract)
            md2 = cp.tile([128, n], f32, name=f"md2{part}")
            nc.scalar.activation(md2[:], mx_all[:, sl], AF.Copy, scale=-2.0)
            nc.vector.tensor_tensor(out=md2[:], in0=md2[:], in1=y_all[:],
                                    op=ALU.add)
            nc.vector.tensor_scalar_max(md2[:], md2[:], 1e-12)
            nc.scalar.sqrt(res_sb[:, lo:hi], md2[:])
            rq = cp.tile([128, n], f32, name=f"rq{part}")
            nc.vector.reciprocal(rq[:], sq_all[:, sl])
            rn_all = cp.tile([128, n], f32, name=f"rn{part}")
            nc.scalar.sqrt(rn_all[:], rq[:])
            rn0_ps = pwp.tile([128, 512], f32, space="PSUM", tag="wk",
                              name=f"rn0ps{part}")
            nc.tensor.matmul(rn0_ps[:, 0:n], lhsT=c_w0[:], rhs=rn_all[:],
                             start=True, stop=True)
            rn0_all = cp.tile([128, n], f32, name=f"rn0{part}")
            nc.scalar.copy(rn0_all[:], rn0_ps[:, 0:n])
            g0m = cp.tile([128, n * CH], f32, name=f"g0m{part}")
            nc.vector.tensor_tensor(out=g0m[:], in0=g0c_all[:, CH * lo:CH * hi],
                                    in1=c_m0p[:, CH * lo:CH * hi], op=ALU.mult)
            g0_all = cp.tile([128, n], f32, name=f"g0a{part}")
            nc.vector.tensor_reduce(
                g0_all[:], g0m[:].rearrange("p (g a) -> p g a", a=CH),
                axis=mybir.AxisListType.X, op=ALU.add)
            cosa = cp.tile([128, n], f32, name=f"cosa{part}")
            nc.vector.tensor_tensor(out=cosa[:], in0=g0_all[:], in1=rn_all[:],
                                    op=ALU.mult)
            nc.vector.tensor_tensor(out=res_sb[:, NG + lo:NG + hi],
                                    in0=cosa[:], in1=rn0_all[:], op=ALU.mult)
            dvm = cp.tile([128, n * K], f32, name=f"dvm{part}")
            nc.vector.tensor_tensor(out=dvm[:], in0=dot_all[:, K * lo:K * hi],
                                    in1=c_mdot[:, K * lo:K * hi], op=ALU.mult)
            dv_all = cp.tile([128, n], f32, name=f"dva{part}")
            nc.vector.tensor_reduce(
                dv_all[:], dvm[:].rearrange("p (g a) -> p g a", a=K),
                axis=mybir.AxisListType.X, op=ALU.add)
            nc.vector.tensor_tensor(out=res_sb[:, 2 * NG + lo:2 * NG + hi],
                                    in0=dv_all[:], in1=rn_all[:], op=ALU.mult)

        # ---- prefetch first groups around phase A ----
        tps = {}
        for g in (0, 1):
            tps[g] = emit_gather_transpose(g)

        # ---- phase A: sentence embeddings ----
        sent_ps = [pwp.tile([16, 512], f32, space="PSUM", tag="wk",
                            name=f"sent{hh}") for hh in range(2)]
        for c in range(NCAP):
            cap = cxp.tile([128, D], bf16, tag="cg", name=f"cap{c}")
            nc.gpsimd.indirect_dma_start(
                out=cap[:], out_offset=None, in_=table[:],
                in_offset=bass.IndirectOffsetOnAxis(
                    ap=c_idxc[:, c:c + 1], axis=0),
            )
            for hh in range(2):
                nc.tensor.matmul(
                    sent_ps[hh][:],
                    lhsT=c_maskB[:, BL * c:BL * (c + 1)],
                    rhs=cap[:, 512 * hh:512 * (hh + 1)],
                    start=(c == 0), stop=(c == NCAP - 1),
                )
        for g in (2, 3):
            tps[g] = emit_gather_transpose(g)
        sent_f = cp.tile([16, 1024], f32)
        nc.scalar.copy(sent_f[:, 0:512], sent_ps[0][:])
        nc.vector.tensor_copy(sent_f[:, 512:1024], sent_ps[1][:])
        sq_scr = cp.tile([16, 1024], f32)
        ssq = cp.tile([16, 1], f32)
        nc.scalar.activation(sq_scr[:], sent_f[:], AF.Square,
                             accum_out=ssq[:])
        ssq2 = cp.tile([16, 1], f32)
        nc.vector.tensor_scalar_max(ssq2[:], ssq[:], 1e-16)
        rss = cp.tile([16, 1], f32)
        nc.vector.reciprocal(rss[:], ssq2[:])
        rsent = cp.tile([16, 1], f32)
        nc.scalar.sqrt(rsent[:], rss[:])
        sentnb = cp.tile([16, 1024], bf16)
        nc.vector.tensor_scalar_mul(sentnb[:], sent_f[:], rsent[:])

        # sentence columns -> [128, 16] chunks via PE transpose, into all tb
        stp = pwp.tile([128, 256], bf16, space="PSUM", tag="wk", name="stp")
        for c in range(CH):
            nc.tensor.transpose(
                stp[:, K * c:K * (c + 1)],
                sentnb[:16, 128 * c:128 * (c + 1)],
                c_i128b[:16, :16],
            )
        tb_tiles = []
        for i in range(3):
            tbt = cp.tile([128, CH * CW], bf16, name=f"tb{i}")
            tb_tiles.append(tbt)
            dst = tbt[:].rearrange("p (c w) -> p c w", w=CW)[:, :, 128:128 + K]
            stps = stp[:, 0:CH * K].rearrange("p (c w) -> p c w", w=K)
            nc.vector.tensor_copy(dst, stps)

        # ---- phase B: steady-state pipeline ----
        PRE = 4
        for g in range(NG):
            emit_group(g, tps.pop(g))
            if g + PRE < NG:
                tps[g + PRE] = emit_gather_transpose(g + PRE)
            if g == 16:
                emit_tail(0, 17, 0)
        emit_tail(17, NG, 1)

        nc.sync.dma_start(res_d[:], res_sb[:])

    nc.compile()
    return nc


def _get_nc():
    if "nc" not in _CACHE:
        _CACHE["nc"] = _build_nc()
    return _CACHE["nc"]


# ---------------- host-side preparation ----------------

def _host_consts():
    import ml_dtypes
    f = np.float32
    bf = ml_dtypes.bfloat16
    i128 = np.eye(128, dtype=f)
    i128b = np.eye(128).astype(bf)
    ineg = (-0.5 * np.eye(128)).astype(bf)
    blk = np.kron(np.eye(PPG), np.ones((K, K))).astype(f)  # block diagonal
    cmask = (-0.5 * LARGE * (1.0 - blk + np.eye(128))).astype(bf)
    # w0[q, m] = 1 iff q == K*(m//K)
    w0 = np.zeros((128, 128), f)
    m = np.arange(128)
    w0[(m // K) * K, m] = 1.0
    # m0p[r, cb] = (cb == r//K) * (r % K != 0), replicated per group
    r = np.arange(128)
    m0p1 = np.zeros((128, CH), f)
    m0p1[r, r // K] = (r % K != 0).astype(f)
    m0p = np.tile(m0p1, (1, NG))
    # mdot_h[r, c] = 1 iff c == 8h + r//K with h = g % 2
    mdot = np.zeros((128, NG * K), f)
    for g in range(NG):
        hh = g % 2
        mdot[r, K * g + 8 * hh + r // K] = 1.0
    return i128, i128b, ineg, cmask, w0, m0p, mdot


def _core_inputs(topk, cap, cap_len, table_np):
    """Build the per-core in_maps for run_bass_kernel_spmd."""
    import ml_dtypes
    bf = ml_dtypes.bfloat16
    i128, i128b, ineg, cmask, w0, m0p, mdot = _host_consts()
    in_maps = []
    for m in range(NCORES):
        bsl = slice(m * BL, (m + 1) * BL)
        tk = topk[:, bsl, :].astype(np.int64)          # [T, BL, K]
        cp_ = cap[bsl].astype(np.int64)                # [BL, L]
        cl = cap_len[bsl].astype(np.int64)             # [BL]

        idx_flat = tk.reshape(-1).astype(np.int32)     # [T*BL*K] = NG*128
        idx_topk = np.ascontiguousarray(
            idx_flat.reshape(NG, 128).T).astype(np.int32)  # [128, NG]

        cap_pad = np.zeros((BL, LPAD), np.int32)
        cap_pad[:, :L] = cp_.astype(np.int32)
        idx_cap = np.ascontiguousarray(
            cap_pad.reshape(-1).reshape(NCAP, 128).T).astype(np.int32)

        # maskB[row, col]: chunk c rows = 32a + l (a in 0..3), col = BL*c + 4c + a
        maskB = np.zeros((128, NCAP * BL), np.float32)
        for c in range(NCAP):
            for a in range(128 // LPAD):
                b = (128 // LPAD) * c + a
                ll = np.arange(LPAD)
                maskB[LPAD * a + ll, BL * c + b] = (ll < cl[b]).astype(
                    np.float32)

        in_maps.append({
            "table": table_np,
            "idx_topk": idx_topk,
            "idx_cap": idx_cap,
            "maskB": maskB.astype(bf),
            "i128": i128, "i128b": i128b, "ineg": ineg, "cmask": cmask,
            "w0": w0, "m0p": m0p, "mdot": mdot,
        })
    return in_maps


def _postprocess(results):
    """results: list of 8 dicts with 'res' [128, 3*NG] -> 3 arrays [B, T, K]."""
    per_core = []
    for m in range(NCORES):
        res = np.asarray(results[m]["res"])            # [128, 3*NG]
        r5 = res.reshape(PPG, K, 3, NG)                # [p_ig, i, o, g]
        r5 = r5.transpose(2, 3, 0, 1)                  # [o, g, p_ig, i]
        r5 = r5.reshape(3, NG * PPG, K)                # [o, p, i], p = t*BL+b
        r5 = r5.reshape(3, T, BL, K)                   # [o, t, b_loc, i]
        per_core.append(r5)
    full = np.concatenate([pc[:, :, None, :, :] for pc in per_core],
                          axis=2)                      # [3, T, m, b_loc, K]
    full = full.reshape(3, T, B, K).transpose(0, 2, 1, 3)  # [3, B, T, K]
    return full[0], full[1], full[2]


def _run(in_maps, trace=False, **kwargs):
    from concourse.bass_utils import run_bass_kernel_spmd
    nc = _get_nc()
    return run_bass_kernel_spmd(
        nc, in_maps, core_ids=list(range(NCORES)), trace=trace, **kwargs)


def kernel(topk_words, caption, cap_len, cap_embedding, _trace=False):
    topk = np.asarray(topk_words)
    cap = np.asarray(caption)
    cl = np.asarray(cap_len)
    table_np = np.ascontiguousarray(np.asarray(cap_embedding,
                                               dtype=np.float32))
    in_maps = _core_inputs(topk, cap, cl, table_np)
    br = _run(in_maps, trace=_trace)
    out = _postprocess(br.results)
    if _trace:
        kernel.last_results = br
    return out


# revision 16
# speedup vs baseline: 1.2053x; 1.2053x over previous
"""Trainium2 Bass kernel for nn_Base_75265006895876 (retrieval_knn).

Data-parallel over batch B=128 -> 16 per core on 8 NeuronCores; the
cap_embedding table is replicated per core.  Per core, (t, b) pairs are
processed in groups of 8 pairs (= 128 gathered embedding rows):

  indirect-DMA gather rows (fp32 -> bf16 cast in DMA)
  -> 8x HWDGE xbar dma_start_transpose (SBUF->SBUF, D onto partitions)
  -> PE gram+dots matmuls (bf16, fp32 accum), fused masking matmuls
  -> cheap per-group column extractions into [128, NGROUPS] tiles
  -> one batched tail: norms / min-distance / cosine chains
  -> results tile -> single DMA out
"""

import sys

if "/opt/trn_rl_repo" not in sys.path:
    sys.path.insert(0, "/opt/trn_rl_repo")

import numpy as np

# ---- problem constants (hardcoded; kernel.py must be self-contained) ----
T, B, K, L, V, D = 17, 128, 16, 24, 30000, 1024
NCORES = 8
BL = B // NCORES              # 16 local batch rows per core
PAIRS = T * BL                # 272 (t, b) pairs per core
NG = PAIRS * K // 128         # 34 groups of 128 gathered rows
PPG = 128 // K                # 8 pairs per group
LPAD = 32                     # caption length padded 24 -> 32
NCAP = BL * LPAD // 128       # 4 caption gather groups
CH = D // 128                 # 8 contraction chunks of 128
CW = 128 + K                  # 144: chunk cols + sentence cols in tb
LARGE = 1.0e6

_CACHE = {}


def _build_nc():
    from concourse import bass, bacc, mybir

    f32 = mybir.dt.float32
    bf16 = mybir.dt.bfloat16
    AF = mybir.ActivationFunctionType
    ALU = mybir.AluOpType

    nc = bacc.Bacc("TRN2", debug=False)

    NM = NG // 2                  # 17 macro groups of 2
    MW = 2 * CW                   # 288 psum cols per macro

    table = nc.dram_tensor("table", [V, D], f32, kind="ExternalInput")
    idx_topk_d = nc.dram_tensor("idx_topk", [128, NG], mybir.dt.int32,
                                kind="ExternalInput")
    idx_cap_d = nc.dram_tensor("idx_cap", [128, NCAP], mybir.dt.int32,
                               kind="ExternalInput")
    maskB_d = nc.dram_tensor("maskB", [128, NCAP * BL], bf16,
                             kind="ExternalInput")
    i128b_d = nc.dram_tensor("i128b", [128, 128], bf16, kind="ExternalInput")
    i2_d = nc.dram_tensor("i2", [128, MW], bf16, kind="ExternalInput")
    ineg_d = nc.dram_tensor("ineg", [128, 128], bf16, kind="ExternalInput")
    cmask_d = nc.dram_tensor("cmask", [128, 128], bf16, kind="ExternalInput")
    w0_d = nc.dram_tensor("w0", [128, 128], f32, kind="ExternalInput")
    m0p_d = nc.dram_tensor("m0p", [128, NG * CH], f32, kind="ExternalInput")
    mdot_d = nc.dram_tensor("mdot", [128, NG * K], f32, kind="ExternalInput")
    zrow_d = nc.dram_tensor("zrow", [1, 512], bf16, kind="ExternalInput")

    res_d = nc.dram_tensor("res", [128, 3 * NG], f32, kind="ExternalOutput")

    from concourse.tile import TileContext
    from contextlib import ExitStack

    with ExitStack() as ctx:
        tc = ctx.enter_context(TileContext(nc))
        cp = ctx.enter_context(tc.tile_pool(name="cp", bufs=1))
        xp = ctx.enter_context(tc.tile_pool(name="xp", bufs=3))
        cxp = ctx.enter_context(tc.tile_pool(name="cxp", bufs=2))
        smp = ctx.enter_context(tc.tile_pool(name="smp", bufs=3))
        ptp = ctx.enter_context(tc.tile_pool(name="ptp", bufs=2, space="PSUM"))
        pwp = ctx.enter_context(tc.tile_pool(name="pwp", bufs=4, space="PSUM"))

        # ---- indices first (gathers depend on them), then constants ----
        c_idxt = cp.tile([128, NG], mybir.dt.int32)
        nc.sync.dma_start(c_idxt[:], idx_topk_d[:])
        c_idxc = cp.tile([128, NCAP], mybir.dt.int32)
        nc.sync.dma_start(c_idxc[:], idx_cap_d[:])
        c_i128b = cp.tile([128, 128], bf16)
        nc.scalar.dma_start(c_i128b[:], i128b_d[:])
        c_maskB = cp.tile([128, NCAP * BL], bf16)
        nc.sync.dma_start(c_maskB[:], maskB_d[:])
        c_ineg = cp.tile([128, 128], bf16)
        nc.scalar.dma_start(c_ineg[:], ineg_d[:])
        c_cm = cp.tile([128, 128], bf16)
        nc.sync.dma_start(c_cm[:], cmask_d[:])
        c_i2 = cp.tile([128, MW], bf16)
        nc.scalar.dma_start(c_i2[:], i2_d[:])
        c_w0 = cp.tile([128, 128], f32)
        nc.sync.dma_start(c_w0[:], w0_d[:])
        c_m0p = cp.tile([128, NG * CH], f32)
        nc.scalar.dma_start(c_m0p[:], m0p_d[:])
        c_mdot = cp.tile([128, NG * K], f32)
        nc.sync.dma_start(c_mdot[:], mdot_d[:])
        c_zrow = cp.tile([1, 512], bf16)
        nc.scalar.dma_start(c_zrow[:], zrow_d[:])

        res_sb = cp.tile([128, 3 * NG], f32)

        # per-group accumulation tiles for the batched tail
        sq_all = cp.tile([128, NG], f32)
        sqb_all = cp.tile([128, NG], bf16)
        mx_all = cp.tile([128, NG], f32)
        g0c_all = cp.tile([128, NG * CH], f32)
        dot_all = cp.tile([128, NG * K], f32)
        tb_tiles = []

        # ---- emission helpers ----
        def emit_gather_transpose(m):
            """stage 1: gather 2 groups, transpose 16 chunks."""
            x = xp.tile([128, 2 * D], bf16, tag="xg", name=f"x{m}")
            for g2 in range(2):
                nc.gpsimd.indirect_dma_start(
                    out=x[:, D * g2:D * (g2 + 1)], out_offset=None,
                    in_=table[:],
                    in_offset=bass.IndirectOffsetOnAxis(
                        ap=c_idxt[:, 2 * m + g2:2 * m + g2 + 1], axis=0),
                )
            tp = ptp.tile([128, 2 * D], bf16, space="PSUM", tag="tp",
                          name=f"tp{m}")
            for c in range(CH):
                for g2 in range(2):
                    nc.tensor.transpose(
                        tp[:, 1024 * g2 + 128 * c:1024 * g2 + 128 * (c + 1)],
                        x[:, D * g2 + 128 * c:D * g2 + 128 * (c + 1)],
                        c_i128b[:],
                    )
            return tp

        def emit_group(m, tp):
            """stage 2: copies, gram+dots matmuls, extractions, masking."""
            tb = tb_tiles[m % 3]
            # tb chunk-c window: [g0 128 | g0 sent 16 | g1 128 | g1 sent 16]
            dst = tb[:].rearrange("p (c w) -> p c w", w=MW)
            src = tp[:].rearrange("p (g c w) -> p c g w", g=2, w=128)
            nc.vector.tensor_copy(dst[:, 0:5, 0:128], src[:, 0:5, 0:1, :])
            nc.vector.tensor_copy(dst[:, 0:5, 144:272], src[:, 0:5, 1:2, :])
            nc.scalar.copy(dst[:, 5:, 0:128], src[:, 5:, 0:1, :])
            nc.scalar.copy(dst[:, 5:, 144:272], src[:, 5:, 1:2, :])

            wk = pwp.tile([128, 512], f32, space="PSUM", tag="wk",
                          name=f"wk{m}")
            # zero the whole bank once; all later matmuls accumulate
            nc.tensor.matmul(wk[:, 0:512], lhsT=c_zrow[:, 0:128],
                             rhs=c_zrow[:], start=True, stop=False,
                             skip_group_check=True)
            for g2 in range(2):
                for c in range(CH):
                    nc.tensor.matmul(
                        wk[:, CW * g2:CW * g2 + CW],
                        lhsT=tb[:, MW * c + CW * g2:MW * c + CW * g2 + 128],
                        rhs=tb[:, MW * c + CW * g2:MW * c + CW * (g2 + 1)],
                        start=False, stop=False,
                        skip_group_check=True,
                    )
            gsl = slice(2 * m, 2 * m + 2)
            # sq = diag(gram) == row max of the unpolluted gram+dots window
            nc.vector.tensor_reduce(
                sq_all[:, gsl], wk[:, 0:MW].rearrange("p (g w) -> p g w", w=CW),
                axis=mybir.AxisListType.X, op=ALU.max)
            nc.vector.tensor_copy(sqb_all[:, gsl], sq_all[:, gsl])
            gv = wk[:, 0:MW].rearrange("p (g a b) -> p g a b", g=2, b=K)
            nc.scalar.copy(
                g0c_all[:, CH * 2 * m:CH * 2 * (m + 1)].rearrange(
                    "p (g a) -> p g a", g=2).unsqueeze(3),
                gv[:, :, 0:CH, 0:1])
            nc.scalar.copy(
                dot_all[:, K * 2 * m:K * 2 * (m + 1)].rearrange(
                    "p (g a) -> p g a", g=2),
                gv[:, :, CH, :])
            for g2 in range(2):
                nc.tensor.matmul(
                    wk[:, CW * g2:CW * g2 + 128],
                    lhsT=sqb_all[:, 2 * m + g2:2 * m + g2 + 1].to_broadcast(
                        [128, 128]),
                    rhs=c_ineg[:], start=False, stop=False,
                    skip_group_check=True,
                )
            nc.tensor.matmul(
                wk[:, 0:MW], lhsT=c_cm[:], rhs=c_i2[:],
                start=False, stop=True, skip_group_check=True,
            )
            nc.vector.tensor_reduce(
                mx_all[:, gsl],
                wk[:, 0:MW].rearrange("p (g w) -> p g w", w=CW)[:, :, 0:128],
                axis=mybir.AxisListType.X, op=ALU.max)

        def emit_tail(lo, hi, part):
            n = hi - lo
            sl = slice(lo, hi)
            # y = 2*sq - bf16(sq): exact cancellation for duplicate rows
            sqbf = cp.tile([128, n], f32, name=f"sqbf{part}")
            nc.vector.tensor_copy(sqbf[:], sqb_all[:, sl])
            y_all = cp.tile([128, n], f32, name=f"y{part}")
            nc.vector.tensor_scalar(out=y_all[:], in0=sq_all[:, sl],
                                    scalar1=2.0, scalar2=None, op0=ALU.mult)
            nc.vector.tensor_tensor(out=y_all[:], in0=y_all[:], in1=sqbf[:],
                                    op=ALU.subtract)
            md2 = cp.tile([128, n], f32, name=f"md2{part}")
            nc.scalar.activation(md2[:], mx_all[:, sl], AF.Copy, scale=-2.0)
            nc.vector.tensor_tensor(out=md2[:], in0=md2[:], in1=y_all[:],
                                    op=ALU.add)
            nc.vector.tensor_scalar_max(md2[:], md2[:], 1e-12)
            nc.scalar.sqrt(res_sb[:, lo:hi], md2[:])
            rq = cp.tile([128, n], f32, name=f"rq{part}")
            nc.vector.reciprocal(rq[:], sq_all[:, sl])
            rn_all = cp.tile([128, n], f32, name=f"rn{part}")
            nc.scalar.sqrt(rn_all[:], rq[:])
            rn0_ps = pwp.tile([128, 512], f32, space="PSUM", tag="wk",
                              name=f"rn0ps{part}")
            nc.tensor.matmul(rn0_ps[:, 0:n], lhsT=c_w0[:], rhs=rn_all[:],
                             start=True, stop=True)
            rn0_all = cp.tile([128, n], f32, name=f"rn0{part}")
            nc.scalar.copy(rn0_all[:], rn0_ps[:, 0:n])
            g0m = cp.tile([128, n * CH], f32, name=f"g0m{part}")
            nc.vector.tensor_tensor(out=g0m[:], in0=g0c_all[:, CH * lo:CH * hi],
                                    in1=c_m0p[:, CH * lo:CH * hi], op=ALU.mult)
            g0_all = cp.tile([128, n], f32, name=f"g0a{part}")
            nc.vector.tensor_reduce(
                g0_all[:], g0m[:].rearrange("p (g a) -> p g a", a=CH),
                axis=mybir.AxisListType.X, op=ALU.add)
            cosa = cp.tile([128, n], f32, name=f"cosa{part}")
            nc.vector.tensor_tensor(out=cosa[:], in0=g0_all[:], in1=rn_all[:],
                                    op=ALU.mult)
            nc.vector.tensor_tensor(out=res_sb[:, NG + lo:NG + hi],
                                    in0=cosa[:], in1=rn0_all[:], op=ALU.mult)
            dvm = cp.tile([128, n * K], f32, name=f"dvm{part}")
            nc.vector.tensor_tensor(out=dvm[:], in0=dot_all[:, K * lo:K * hi],
                                    in1=c_mdot[:, K * lo:K * hi], op=ALU.mult)
            dv_all = cp.tile([128, n], f32, name=f"dva{part}")
            nc.vector.tensor_reduce(
                dv_all[:], dvm[:].rearrange("p (g a) -> p g a", a=K),
                axis=mybir.AxisListType.X, op=ALU.add)
            nc.vector.tensor_tensor(out=res_sb[:, 2 * NG + lo:2 * NG + hi],
                                    in0=dv_all[:], in1=rn_all[:], op=ALU.mult)

        # ---- prefetch first macros around phase A ----
        tps = {0: emit_gather_transpose(0)}

        # ---- phase A: sentence embeddings ----
        sent_ps = [pwp.tile([16, 512], f32, space="PSUM", tag="wk",
                            name=f"sent{hh}") for hh in range(2)]
        for c in range(NCAP):
            cap = cxp.tile([128, D], bf16, tag="cg", name=f"cap{c}")
            nc.gpsimd.indirect_dma_start(
                out=cap[:], out_offset=None, in_=table[:],
                in_offset=bass.IndirectOffsetOnAxis(
                    ap=c_idxc[:, c:c + 1], axis=0),
            )
            for hh in range(2):
                nc.tensor.matmul(
                    sent_ps[hh][:],
                    lhsT=c_maskB[:, BL * c:BL * (c + 1)],
                    rhs=cap[:, 512 * hh:512 * (hh + 1)],
                    start=(c == 0), stop=(c == NCAP - 1),
                )
        tps[1] = emit_gather_transpose(1)
        sent_f = cp.tile([16, 1024], f32)
        nc.scalar.copy(sent_f[:, 0:512], sent_ps[0][:])
        nc.vector.tensor_copy(sent_f[:, 512:1024], sent_ps[1][:])
        sq_scr = cp.tile([16, 1024], f32)
        ssq = cp.tile([16, 1], f32)
        nc.scalar.activation(sq_scr[:], sent_f[:], AF.Square,
                             accum_out=ssq[:])
        ssq2 = cp.tile([16, 1], f32)
        nc.vector.tensor_scalar_max(ssq2[:], ssq[:], 1e-16)
        rss = cp.tile([16, 1], f32)
        nc.vector.reciprocal(rss[:], ssq2[:])
        rsent = cp.tile([16, 1], f32)
        nc.scalar.sqrt(rsent[:], rss[:])
        sentnb = cp.tile([16, 1024], bf16)
        nc.vector.tensor_scalar_mul(sentnb[:], sent_f[:], rsent[:])

        # sentence columns -> [128, 16] chunks via PE transpose, into all tb
        stp = pwp.tile([128, 256], bf16, space="PSUM", tag="wk", name="stp")
        for c in range(CH):
            nc.tensor.transpose(
                stp[:, K * c:K * (c + 1)],
                sentnb[:16, 128 * c:128 * (c + 1)],
                c_i128b[:16, :16],
            )
        for i in range(3):
            tbt = cp.tile([128, CH * MW], bf16, name=f"tb{i}")
            tb_tiles.append(tbt)
            stps = stp[:, 0:CH * K].rearrange("p (c w) -> p c w", w=K)
            for g2 in range(2):
                dst = tbt[:].rearrange("p (c w) -> p c w", w=MW)[
                    :, :, CW * g2 + 128:CW * g2 + 144]
                nc.vector.tensor_copy(dst, stps)

        # ---- phase B: steady-state pipeline ----
        NM_PRE = 2
        for m in range(NM):
            emit_group(m, tps.pop(m))
            if m + NM_PRE < NM:
                tps[m + NM_PRE] = emit_gather_transpose(m + NM_PRE)
            if m == 8:
                emit_tail(0, 18, 0)
        emit_tail(18, NG, 1)

        nc.sync.dma_start(res_d[:], res_sb[:])

    nc.compile()
    return nc


def _get_nc():
    if "nc" not in _CACHE:
        _CACHE["nc"] = _build_nc()
    return _CACHE["nc"]


# ---------------- host-side preparation ----------------

def _host_consts():
    import ml_dtypes
    f = np.float32
    bf = ml_dtypes.bfloat16
    i128b = np.eye(128).astype(bf)
    i2 = np.zeros((128, 2 * (128 + K)), np.float32)
    i2[:, 0:128] = np.eye(128)
    i2[:, 128 + K:2 * 128 + K] = np.eye(128)
    i2 = i2.astype(bf)
    ineg = (-0.5 * np.eye(128)).astype(bf)
    blk = np.kron(np.eye(PPG), np.ones((K, K))).astype(f)  # block diagonal
    cmask = (-0.5 * LARGE * (1.0 - blk + np.eye(128))).astype(bf)
    # w0[q, m] = 1 iff q == K*(m//K)
    w0 = np.zeros((128, 128), f)
    m = np.arange(128)
    w0[(m // K) * K, m] = 1.0
    # m0p[r, cb] = (cb == r//K) * (r % K != 0), replicated per group
    r = np.arange(128)
    m0p1 = np.zeros((128, CH), f)
    m0p1[r, r // K] = (r % K != 0).astype(f)
    m0p = np.tile(m0p1, (1, NG))
    # mdot_h[r, c] = 1 iff c == 8h + r//K with h = g % 2
    mdot = np.zeros((128, NG * K), f)
    for g in range(NG):
        hh = g % 2
        mdot[r, K * g + 8 * hh + r // K] = 1.0
    zrow = np.zeros((1, 512), np.float32).astype(bf)
    return i128b, i2, ineg, cmask, w0, m0p, mdot, zrow


def _core_inputs(topk, cap, cap_len, table_np):
    """Build the per-core in_maps for run_bass_kernel_spmd."""
    import ml_dtypes
    bf = ml_dtypes.bfloat16
    i128b, i2, ineg, cmask, w0, m0p, mdot, zrow = _host_consts()
    in_maps = []
    for m in range(NCORES):
        bsl = slice(m * BL, (m + 1) * BL)
        tk = topk[:, bsl, :].astype(np.int64)          # [T, BL, K]
        cp_ = cap[bsl].astype(np.int64)                # [BL, L]
        cl = cap_len[bsl].astype(np.int64)             # [BL]

        idx_flat = tk.reshape(-1).astype(np.int32)     # [T*BL*K] = NG*128
        idx_topk = np.ascontiguousarray(
            idx_flat.reshape(NG, 128).T).astype(np.int32)  # [128, NG]

        cap_pad = np.zeros((BL, LPAD), np.int32)
        cap_pad[:, :L] = cp_.astype(np.int32)
        idx_cap = np.ascontiguousarray(
            cap_pad.reshape(-1).reshape(NCAP, 128).T).astype(np.int32)

        # maskB[row, col]: chunk c rows = 32a + l (a in 0..3), col = BL*c + 4c + a
        maskB = np.zeros((128, NCAP * BL), np.float32)
        for c in range(NCAP):
            for a in range(128 // LPAD):
                b = (128 // LPAD) * c + a
                ll = np.arange(LPAD)
                maskB[LPAD * a + ll, BL * c + b] = (ll < cl[b]).astype(
                    np.float32)

        in_maps.append({
            "table": table_np,
            "idx_topk": idx_topk,
            "idx_cap": idx_cap,
            "maskB": maskB.astype(bf),
            "i128b": i128b, "i2": i2, "ineg": ineg, "cmask": cmask,
            "w0": w0, "m0p": m0p, "mdot": mdot, "zrow": zrow,
        })
    return in_maps


def _postprocess(results):
    """results: list of 8 dicts with 'res' [128, 3*NG] -> 3 arrays [B, T, K]."""
    per_core = []
    for m in range(NCORES):
        res = np.asarray(results[m]["res"])            # [128, 3*NG]
        r5 = res.reshape(PPG, K, 3, NG)                # [p_ig, i, o, g]
        r5 = r5.transpose(2, 3, 0, 1)                  # [o, g, p_ig, i]
        r5 = r5.reshape(3, NG * PPG, K)                # [o, p, i], p = t*BL+b
        r5 = r5.reshape(3, T, BL, K)                   # [o, t, b_loc, i]
        per_core.append(r5)
    full = np.concatenate([pc[:, :, None, :, :] for pc in per_core],
                          axis=2)                      # [3, T, m, b_loc, K]
    full = full.reshape(3, T, B, K).transpose(0, 2, 1, 3)  # [3, B, T, K]
    return full[0], full[1], full[2]


def _run(in_maps, trace=False, **kwargs):
    from concourse.bass_utils import run_bass_kernel_spmd
    nc = _get_nc()
    return run_bass_kernel_spmd(
        nc, in_maps, core_ids=list(range(NCORES)), trace=trace, **kwargs)


def kernel(topk_words, caption, cap_len, cap_embedding, _trace=False):
    topk = np.asarray(topk_words)
    cap = np.asarray(caption)
    cl = np.asarray(cap_len)
    table_np = np.ascontiguousarray(np.asarray(cap_embedding,
                                               dtype=np.float32))
    in_maps = _core_inputs(topk, cap, cl, table_np)
    br = _run(in_maps, trace=_trace)
    out = _postprocess(br.results)
    if _trace:
        kernel.last_results = br
    return out
